# revision 1
# baseline (speedup 1.0000x reference)
"""Trainium2 Bass kernel for nn_DeformableConvLSTMCell_33895881900284.

Full (unsharded) inputs in, full outputs out. Internally: data-parallel over
batch across 8 NeuronCores (8 batches per core), conv weights / gate params
replicated.

Math per the reference:
  outI  = conv3x3_same(inputs, wconvInput)
  g     = tanh(outI + conv3x3_same(hidden_prev, wconvHidden) + gateBias)
  gapI  = mean_hw(outI);  gapH = mean_hw(hidden_prev)          # [B, D]
  i/f/o = sigmoid(wx*gapI + wh*gapH + bias)                    # [B, D]
  tiled gate: value used at (b, h, w, c) is gate[(28*b + h) % 64, c]
  state  = f*state_prev + i*g;  hidden = o*tanh(state)

The (28*b+h)%64 scrambling makes gates cross-batch: each core computes its
local GAP columns, all cores AllGather them, and a per-core index-array input
drives an indirect-DMA gather of exactly the gate rows this core's outputs
need (the SPMD program stays identical across cores; only input data differs).

gapI never touches the conv output. By linearity, 784*gapI is a combination
of 9 masked pixel sums of the raw input (full sum, edge rows/cols, corners)
matmul'd with summed conv-weight taps. Stage A computes those masked sums
with tiny fp32 matmuls on natural-layout tiles (mask vectors as the moving
operand), so the AllGather fires ~50us into the kernel and the gate tables
are ready long before the first elementwise consumer. Stage B then runs a
single fused per-batch pipeline: PE transposes inputs/hidden/state_prev to
[channel, pixel] layout, 3x3 conv = 36 shifted float32r matmuls + a gateBias
identity-matmul accumulating in one PSUM bank per window, ACT applies tanh,
GpSimd applies the gates (broadcast via stride-0 APs), PE transposes the
results back, and DMA stores natural-layout outputs.

float32r = full-rate fp32 matmul with TF32-like operand rounding; operands
are produced by DVE copies with float32r output dtype.
"""
import numpy as np

import bass_rust
import concourse.bass as bass
import concourse.mybir as mybir
import concourse.tile as tile
from concourse.bass_utils import run_bass_kernel_spmd

F32 = mybir.dt.float32
F32R = mybir.dt.float32r
I32 = mybir.dt.int32
AF = mybir.ActivationFunctionType
ALU = mybir.AluOpType

N_CORES = 8
B, H, W, CIN, D = 64, 28, 28, 256, 256
BL = B // N_CORES          # local batches per core
PIX = H * W                # 784
PG = 112                   # pixels per transpose group (4 rows)
NPG = PIX // PG            # 7
PAD = 30                   # padded row/col length
XTLEN = PAD * PAD          # 900
NW = 2                     # windows per batch
WROWS = H // NW            # 14
WN = WROWS * W             # 392
NCC = CIN // 128           # 2 channel chunks
NDC = D // 128             # 2 output-channel chunks

# tap order t = 3*kh + kw ; dh = kh-1, dw = kw-1
TAPS = [(kh, kw) for kh in range(3) for kw in range(3)]

# ---------------------------------------------------------------------------
# walrus fixup: split semaphore waits that exceed the per-instruction budget
# (observed: Drain and Matmult accept only 1 semaphore wait each).
MAX_WAITS = 1


def _split_excess_sem_waits(nc):
    counter = [0]
    for fn in nc.m.functions:
        for bb in fn.blocks:
            insts = bb.instructions
            i = 0
            while i < len(insts):
                inst = insts[i]
                si = inst.sync_info
                if si is not None and si.on_wait and len(si.on_wait) > MAX_WAITS:
                    waits = list(si.on_wait)
                    excess = waits[:-MAX_WAITS]
                    keep = waits[-MAX_WAITS:]
                    new_insts = []
                    for j in range(0, len(excess), MAX_WAITS):
                        chunk = excess[j:j + MAX_WAITS]
                        noop = mybir.InstNoOp(
                            name=f"I-waitsplit-{counter[0]}", ins=[], outs=[])
                        counter[0] += 1
                        noop.engine = inst.engine
                        noop.sync_info = bass_rust.SyncInfo(
                            on_wait=chunk, on_update=[])
                        nc.register_instruction(noop)
                        new_insts.append(noop)
                    inst.sync_info = bass_rust.SyncInfo(
                        on_wait=keep, on_update=list(si.on_update))
                    insts[i:i] = new_insts
                    i += len(new_insts)
                i += 1
    return nc


# ---------------------------------------------------------------------------
def _r3(ap, w):
    """view flat free dim as (rows, w)"""
    return ap.rearrange("c (r w) -> c r w", w=w)


def build_nc():
    nc = bass.Bass("TRN2", target_bir_lowering=False, debug=False,
                   num_devices=N_CORES)

    dram = {}
    dram["inputs"] = nc.dram_tensor("inputs", [BL, PIX, CIN], F32,
                                    kind="ExternalInput").ap()
    dram["state_prev"] = nc.dram_tensor("state_prev", [BL, PIX, D], F32,
                                        kind="ExternalInput").ap()
    dram["hidden_prev"] = nc.dram_tensor("hidden_prev", [BL, PIX, D], F32,
                                         kind="ExternalInput").ap()
    dram["w1"] = nc.dram_tensor("wconvInput", [3, 3, CIN, D], F32,
                                kind="ExternalInput").ap()
    dram["w2"] = nc.dram_tensor("wconvHidden", [3, 3, D, D], F32,
                                kind="ExternalInput").ap()
    dram["gb"] = nc.dram_tensor("gateBias", [PIX, D], F32,
                                kind="ExternalInput").ap()
    dram["vec"] = {}
    for nm in ("wxi", "whi", "inputBias", "wxf", "whf", "forgetBias",
               "wxo", "who", "outputBias"):
        dram["vec"][nm] = nc.dram_tensor(nm, [D, 1], F32,
                                         kind="ExternalInput").ap()
    dram["ident"] = nc.dram_tensor("identity", [128, 128], F32,
                                   kind="ExternalInput").ap()
    dram["idx"] = nc.dram_tensor("gate_idx", [4, 224, 1], I32,
                                 kind="ExternalInput").ap()
    dram["masks"] = nc.dram_tensor("gap_masks", [NPG, PG, 9], F32,
                                   kind="ExternalInput").ap()
    dram["hidden"] = nc.dram_tensor("hidden", [BL, NDC, 128, PIX], F32,
                                    kind="ExternalOutput").ap()
    dram["state"] = nc.dram_tensor("state", [BL, NDC, 128, PIX], F32,
                                   kind="ExternalOutput").ap()
    dram["cc_in"] = nc.dram_tensor("cc_in", [32, 128], F32, kind="Internal").ap()
    dram["cc_out"] = nc.dram_tensor("cc_out", [N_CORES * 32, 128], F32,
                                    kind="Internal", addr_space="Shared").ap()

    ctx_mgr = nc.allow_low_precision("float32r operand rounding for PE")
    ctx_mgr.__enter__()
    with tile.TileContext(nc) as tc:
        _build_body(nc, tc, dram)
    ctx_mgr.__exit__(None, None, None)
    return nc


def _build_body(nc, tc, dram):
    from contextlib import ExitStack
    ctx = ExitStack()
    pool = lambda **kw: ctx.enter_context(tc.tile_pool(**kw))

    const = pool(name="const", bufs=1)
    wts = pool(name="wts", bufs=1)
    stage = pool(name="stage", bufs=4)
    natp = pool(name="natp", bufs=10)      # stage-A natural tiles
    xt_in = pool(name="xt_in", bufs=2)
    xt_hid = pool(name="xt_hid", bufs=2)
    xt_sp = pool(name="xt_sp", bufs=3)
    ew = pool(name="ew", bufs=3)
    outb = pool(name="outb", bufs=4)
    gsm = pool(name="gsm", bufs=1)
    gtmp = pool(name="gtmp", bufs=2)
    ps_conv = pool(name="ps_conv", bufs=3, space="PSUM")
    ps_tr = pool(name="ps_tr", bufs=3, space="PSUM")
    ps_gap = pool(name="ps_gap", bufs=1, space="PSUM")

    # ---- minimal constants needed by stage A ----
    ident = const.tile([128, 128], F32, tag="ident")
    nc.sync.dma_start(ident[:], dram["ident"][:])
    masks = []
    for pg in range(NPG):
        m = const.tile([PG, 9], F32, tag=f"mask{pg}", name=f"mask{pg}")
        nc.sync.dma_start(m[:], dram["masks"][pg])
        masks.append(m)
    idx_sb = []
    for g4 in range(4):
        halves = []
        for hf in range(2):
            t = const.tile([PG, 1], I32, tag=f"idx{g4}_{hf}")
            nc.sync.dma_start(t[:], dram["idx"][g4, hf * PG:(hf + 1) * PG, :])
            halves.append(t)
        idx_sb.append(halves)

    raw = [gsm.tile([128, 9 * BL], F32R, tag=f"raw{cc}", name=f"raw{cc}")
           for cc in range(NCC)]
    gapH = [gsm.tile([128, BL], F32, tag=f"gapH{cc}", name=f"gapHs{cc}")
            for cc in range(NCC)]

    # ============ stage B: fused per-batch pipeline =========================
    from collections import deque
    tqueue = deque()   # pending transpose-emitter closures (next batch)

    def drain_tq(n):
        for _ in range(min(n, len(tqueue))):
            tqueue.popleft()()

    def queue_transposed(j, dsrc, xpool, tagbase, dtype, padded):
        """queue DMA+transpose+copy work building [128, 900|784] per cc."""
        tiles = []
        for cc in range(NCC):
            xlen = XTLEN if padded else PIX
            xt = xpool.tile([128, xlen], dtype, tag=f"{tagbase}{cc}",
                            name=f"{tagbase}{cc}_{j}")
            if padded:
                x3 = _r3(xt[:].bitcast(F32), PAD)
                nc.gpsimd.memset(x3[:, 0:1, :], 0.0)
                nc.gpsimd.memset(x3[:, PAD - 1:PAD, :], 0.0)
                nc.gpsimd.memset(x3[:, 1:PAD - 1, 0:1], 0.0)
                nc.gpsimd.memset(x3[:, 1:PAD - 1, PAD - 1:PAD], 0.0)
            tiles.append(xt)

        def emit_group(g7):
            nat = stage.tile([128, 256], F32, tag="natload", name="natload")
            nc.sync.dma_start(nat[0:PG, :], dsrc[j, g7 * PG:(g7 + 1) * PG, :])
            for cc in range(NCC):
                pt = ps_tr.tile([128, PG], F32, tag="ptr", name="pt_tr")
                nc.tensor.transpose(pt[:], nat[0:PG, cc * 128:(cc + 1) * 128],
                                    ident[0:PG, 0:PG])
                if padded:
                    dst = _r3(tiles[cc][:], PAD)[:, 1 + 4 * g7:1 + 4 * g7 + 4,
                                                 1:29]
                else:
                    dst = _r3(tiles[cc][:, g7 * PG:(g7 + 1) * PG], W)
                nc.vector.tensor_copy(dst,
                                      pt[:].rearrange("c (r w) -> c r w", w=W))

        for g7 in range(NPG):
            tqueue.append(lambda g7=g7: emit_group(g7))
        return tiles

    xbufs = {}

    def queue_batch_inputs(j):
        xbufs[j] = (
            queue_transposed(j, dram["inputs"], xt_in, "xin", F32R, True),
            queue_transposed(j, dram["hidden_prev"], xt_hid, "xhid", F32R, True),
            queue_transposed(j, dram["state_prev"], xt_sp, "xsp", F32, False),
        )

    def emit_conv_window(j, wi, dc):
        """conv+bias -> PSUM -> tanh -> gt tile; returns gt"""
        xin, xhid, _ = xbufs[j]
        h0 = 1 + wi * WROWS
        base = (h0 - 1) * W
        p = ps_conv.tile([128, WN], F32, tag="pconv", name="pconv")
        p3 = _r3(p[:], W)
        nc.tensor.matmul(p[:], ident_r[:], gbias[dc][:, base:base + WN],
                         start=True, stop=False)
        for conv, xbuf in ((0, xin), (1, xhid)):
            for t, (kh, kw) in enumerate(TAPS):
                dh, dwid = kh - 1, kw - 1
                for cc in range(NCC):
                    rhs = _r3(xbuf[cc][:], PAD)[
                        :, h0 + dh:h0 + dh + WROWS, 1 + dwid:1 + dwid + W]
                    last = (conv == 1 and t == 8 and cc == NCC - 1)
                    nc.tensor.matmul(
                        p3, wblk(conv, t, cc)[:, dc * 128:(dc + 1) * 128],
                        rhs, start=False, stop=last)
        gt = ew.tile([128, WN], F32, tag="gt", bufs=6, name="gt")
        nc.scalar.activation(gt[:], p[:], AF.Tanh)
        return gt

    def emit_elementwise(j, wi, dc, gt, stT, hidT, gates):
        _, _, xsp = xbufs[j]
        h0 = 1 + wi * WROWS
        base = (h0 - 1) * W
        t0 = j * H + (h0 - 1)

        def gw(gate):
            return gates[gate][dc][:, t0:t0 + WROWS].to_broadcast(
                [128, WROWS, W])

        sp3 = _r3(xsp[dc][:, base:base + WN], W)
        g3 = _r3(gt[:], W)
        st3 = _r3(stT[dc][:, base:base + WN], W)
        hd3 = _r3(hidT[dc][:, base:base + WN], W)
        s1 = ew.tile([128, WN], F32, tag="s1", name="s1")
        nc.gpsimd.tensor_tensor(out=_r3(s1[:], W), in0=sp3, in1=gw("f"),
                                op=ALU.mult)
        s2 = ew.tile([128, WN], F32, tag="s2", name="s2")
        nc.gpsimd.tensor_tensor(out=_r3(s2[:], W), in0=g3, in1=gw("i"),
                                op=ALU.mult)
        nc.vector.tensor_tensor(out=st3, in0=_r3(s1[:], W), in1=_r3(s2[:], W),
                                op=ALU.add)
        th = ew.tile([128, WN], F32, tag="th", name="th")
        nc.scalar.activation(th[:], stT[dc][:, base:base + WN], AF.Tanh)
        nc.gpsimd.tensor_tensor(out=hd3, in0=_r3(th[:], W), in1=gw("o"),
                                op=ALU.mult)

    def emit_store(j, stT, hidT):
        # outputs leave the chip transposed ([dc, 128, pix]); the host
        # reassembles to NHWC during unsharding.
        for dname, buf in (("state", stT), ("hidden", hidT)):
            for dc in range(NDC):
                nc.scalar.dma_start(dram[dname][j, dc], buf[dc][:])

    def out_tiles(j):
        stT = [outb.tile([128, PIX], F32, tag="stT", name=f"stT{j}_{dc}")
               for dc in range(NDC)]
        hidT = [outb.tile([128, PIX], F32, tag="hidT", name=f"hidT{j}_{dc}")
                for dc in range(NDC)]
        return stT, hidT

    queue_batch_inputs(0)
    queue_batch_inputs(1)

    # ============ stage A: masked pixel sums via fp32 matmuls ==============
    # lhsT = mask vectors [112, 9] (9-column weight load: cheap),
    # rhs = natural-layout tiles [112, 256]; out = RAW^T [9, 256] accumulated
    # over the 7 pixel groups, then transposed into [c, 9]/[c, 1] form.
    # ---- remaining constants (overlap the stage-A matmul stream) ----
    ident_r = const.tile([128, 128], F32R, tag="ident_r")
    nc.vector.tensor_copy(ident_r[:], ident[:])
    vecs = {}
    for nm in dram["vec"]:
        t = const.tile([128, NDC], F32, tag=f"vec_{nm}")
        for c in range(NDC):
            nc.scalar.dma_start(t[:, c:c + 1],
                                dram["vec"][nm][c * 128:(c + 1) * 128, :])
        if nm.startswith("wx") or nm.startswith("wh"):
            nc.vector.tensor_scalar_mul(t[:], t[:], 1.0 / PIX)
        vecs[nm] = t

    wconv = wts.tile([128, 2 * 9 * NCC * 256], F32R, tag="wconv")

    def wblk(conv, t, cc):
        off = ((conv * 9 + t) * NCC + cc) * 256
        return wconv[:, off:off + 256]

    for conv, dw in ((0, dram["w1"]), (1, dram["w2"])):
        for t, (kh, kw) in enumerate(TAPS):
            for cc in range(NCC):
                ws = stage.tile([128, 256], F32, tag="wstage")
                nc.scalar.dma_start(ws[:], dw[kh, kw, cc * 128:(cc + 1) * 128, :])
                nc.scalar.copy(wblk(conv, t, cc), ws[:])

    # gateBias transposed to [128 d, PIX] per dc, f32r (conv-PSUM accumuland)
    gbias = [const.tile([128, PIX], F32R, tag=f"gbias{dc}", name=f"gbias{dc}")
             for dc in range(NDC)]
    for g7 in range(NPG):
        nat = stage.tile([128, 256], F32, tag="natload")
        nc.scalar.dma_start(nat[0:PG, :], dram["gb"][g7 * PG:(g7 + 1) * PG, :])
        for dc in range(NDC):
            pt = ps_tr.tile([128, PG], F32, tag="ptr")
            nc.tensor.transpose(pt[:], nat[0:PG, dc * 128:(dc + 1) * 128],
                                ident[0:PG, 0:PG])
            nc.vector.tensor_copy(gbias[dc][:, g7 * PG:(g7 + 1) * PG], pt[:])


    gt0 = {}
    masks_r = []
    for pg in range(NPG):
        mr = const.tile([PG, 9], F32R, tag=f"maskr{pg}", name=f"maskr{pg}")
        nc.vector.tensor_copy(mr[:], masks[pg][:])
        masks_r.append(mr)
    for j in range(BL):
        for tensor, dsrc in (("in", dram["inputs"]), ("hid", dram["hidden_prev"])):
            p9 = ps_gap.tile([9, 256], F32, tag="rawT")
            for pg in range(NPG):
                nat = natp.tile([PG, 256], F32, tag="nat",
                                name=f"nat{tensor}{j}_{pg}")
                nc.sync.dma_start(nat[:], dsrc[j, pg * PG:(pg + 1) * PG, :])
                natr = natp.tile([PG, 256], F32R, tag="natr",
                                 name=f"natr{tensor}{j}_{pg}")
                nc.vector.tensor_copy(natr[:], nat[:])
                nc.tensor.matmul(p9[:], masks_r[pg][:], natr[:],
                                 start=(pg == 0), stop=(pg == NPG - 1))
            rt = gtmp.tile([9, 256], F32, tag="rawT_sb")
            nc.vector.tensor_copy(rt[:], p9[:])
            for cc in range(NCC):
                cs = slice(cc * 128, (cc + 1) * 128)
                pt = ps_tr.tile([128, 9], F32, tag="ptr")
                nc.tensor.transpose(pt[:], rt[:, cs], ident[0:9, 0:9])
                if tensor == "in":
                    nc.vector.tensor_copy(
                        _r3(raw[cc][:], BL)[:, :, j:j + 1],
                        pt[:].rearrange("c (n o) -> c n o", o=1))
                else:
                    nc.vector.tensor_copy(gapH[cc][:, j:j + 1], pt[:, 0:1])
            drain_tq(3)
        if j == 5:
            gt0[(0, 0)] = emit_conv_window(0, 0, 0)
            gt0[(0, 1)] = emit_conv_window(0, 0, 1)
        elif j == 6:
            gt0[(1, 0)] = emit_conv_window(0, 1, 0)
        elif j == 7:
            gt0[(1, 1)] = emit_conv_window(0, 1, 1)

    drain_tq(len(tqueue))

    # combined A-tiles for gapI (conv1 weights); group order:
    # S, Rf, Rl, Cf, Cl, K00, K0L, KL0, KLL (natural coords: Rf=row0, Cl=col27)
    a_r = wts.tile([128, NCC * 9 * 256], F32R, tag="a_r")

    def ablk(cc, g):
        off = (cc * 9 + g) * 256
        return a_r[:, off:off + 256]

    for cc in range(NCC):
        nc.vector.tensor_copy(ablk(cc, 0), wblk(0, 0, cc))
        for t in range(1, 9):
            nc.vector.tensor_tensor(out=ablk(cc, 0), in0=ablk(cc, 0),
                                    in1=wblk(0, t, cc), op=ALU.add)
        for g, taps in ((1, [6, 7, 8]), (2, [0, 1, 2]),
                        (3, [2, 5, 8]), (4, [0, 3, 6])):
            nc.vector.tensor_copy(ablk(cc, g), wblk(0, taps[0], cc))
            for t in taps[1:]:
                nc.vector.tensor_tensor(out=ablk(cc, g), in0=ablk(cc, g),
                                        in1=wblk(0, t, cc), op=ALU.add)
            nc.vector.tensor_scalar_mul(ablk(cc, g), ablk(cc, g), -1.0)
        for g, t in ((5, 8), (6, 6), (7, 2), (8, 0)):
            nc.vector.tensor_copy(ablk(cc, g), wblk(0, t, cc))


    # ---- gapI combine + staging + AllGather launch ----
    gap_ps = ps_gap.tile([8, 256], F32, tag="gapI")
    for cc in range(NCC):
        for g in range(9):
            nc.tensor.matmul(gap_ps[:], raw[cc][:, g * BL:(g + 1) * BL],
                             ablk(cc, g),
                             start=(cc == 0 and g == 0),
                             stop=(cc == NCC - 1 and g == 8))
    gapI_sb = gsm.tile([8, 256], F32, tag="gapI_sb")
    nc.vector.tensor_copy(gapI_sb[:], gap_ps[:])
    nc.sync.dma_start(dram["cc_in"][0:8, :], gapI_sb[:, 0:128])
    nc.sync.dma_start(dram["cc_in"][8:16, :], gapI_sb[:, 128:256])
    for cc in range(NCC):
        pt = ps_gap.tile([8, 128], F32, tag="rawT")
        nc.tensor.transpose(pt[:], gapH[cc][:], ident[:])
        hs = gsm.tile([8, 128], F32, tag=f"gapH_sb{cc}", name=f"gapHsb{cc}")
        nc.vector.tensor_copy(hs[:], pt[:])
        nc.sync.dma_start(dram["cc_in"][16 + 8 * cc:24 + 8 * cc, :], hs[:])

    nc.gpsimd.collective_compute(
        "AllGather", ALU.bypass, replica_groups=[list(range(N_CORES))],
        ins=[dram["cc_in"][:]], outs=[dram["cc_out"][:]])

    # batch 0: inputs + convs first (no gate dependency), gates next,
    # elementwise afterwards - keeps the PE stream from head-of-line
    # blocking on the AllGather.

    # ---- gather + gate tables (waits on AllGather, off the conv path) ----
    sel = [gsm.tile([128, 224], F32, tag=f"sel{g4}", name=f"sel{g4}")
           for g4 in range(4)]
    for g4 in range(4):
        for hf in range(2):
            gtile = stage.tile([PG, 128], F32, tag="gath", name="gath")
            nc.gpsimd.indirect_dma_start(
                out=gtile[:], out_offset=None, in_=dram["cc_out"][:],
                in_offset=bass.IndirectOffsetOnAxis(ap=idx_sb[g4][hf][:, :1],
                                                    axis=0))
            pt = ps_tr.tile([128, PG], F32, tag="ptr", name="pt_gath")
            nc.tensor.transpose(pt[:], gtile[:], ident[0:PG, 0:PG])
            nc.vector.tensor_copy(sel[g4][:, hf * PG:(hf + 1) * PG], pt[:])

    gates = {}
    for gate, wx, wh, bi in (("i", "wxi", "whi", "inputBias"),
                             ("f", "wxf", "whf", "forgetBias"),
                             ("o", "wxo", "who", "outputBias")):
        per_dc = []
        for dc in range(NDC):
            t1 = gtmp.tile([128, 224], F32, tag="gm1", name="gm1")
            nc.vector.tensor_scalar_mul(t1[:], sel[dc][:],
                                        vecs[wx][:, dc:dc + 1])
            t2 = gtmp.tile([128, 224], F32, tag="gm2", name="gm2")
            nc.vector.tensor_scalar_mul(t2[:], sel[2 + dc][:],
                                        vecs[wh][:, dc:dc + 1])
            nc.vector.tensor_tensor(out=t1[:], in0=t1[:], in1=t2[:], op=ALU.add)
            gt = gsm.tile([128, 224], F32, tag=f"gate_{gate}{dc}",
                          name=f"gate_{gate}{dc}")
            nc.scalar.activation(gt[:], t1[:], AF.Sigmoid,
                                 bias=vecs[bi][:, dc:dc + 1])
            per_dc.append(gt)
        gates[gate] = per_dc

    # batch 0 elementwise + store
    stT, hidT = out_tiles(0)
    for wi in range(NW):
        for dc in range(NDC):
            emit_elementwise(0, wi, dc, gt0[(wi, dc)], stT, hidT, gates)
            drain_tq(3)
    emit_store(0, stT, hidT)

    # batches 1..7
    for j in range(1, BL):
        if j + 1 < BL:
            queue_batch_inputs(j + 1)
        drain_tq(6)
        stT, hidT = out_tiles(j)
        for wi in range(NW):
            for dc in range(NDC):
                gt = emit_conv_window(j, wi, dc)
                emit_elementwise(j, wi, dc, gt, stT, hidT, gates)
                drain_tq(4 if j + 1 < BL else len(tqueue))
        emit_store(j, stT, hidT)
    drain_tq(len(tqueue))

    ctx.close()


# ---------------------------------------------------------------------------
_NC_CACHE = None


def _get_nc():
    global _NC_CACHE
    if _NC_CACHE is None:
        nc = build_nc()
        _split_excess_sem_waits(nc)
        _NC_CACHE = nc
    return _NC_CACHE


def _gate_idx(core):
    idx = np.empty((4, 224, 1), np.int32)
    for j in range(BL):
        for hh in range(H):
            t = j * H + hh
            sel_b = (H * (BL * core + j) + hh) % B
            cp, bp = sel_b // BL, sel_b % BL
            for g in range(4):
                idx[g, t, 0] = cp * 32 + g * 8 + bp
    return idx


def _gap_masks():
    m = np.zeros((PIX, 9), np.float32)
    hw = np.arange(PIX)
    r, c = hw // W, hw % W
    m[:, 0] = 1.0
    m[r == 0, 1] = 1.0
    m[r == H - 1, 2] = 1.0
    m[c == 0, 3] = 1.0
    m[c == W - 1, 4] = 1.0
    m[(r == 0) & (c == 0), 5] = 1.0
    m[(r == 0) & (c == W - 1), 6] = 1.0
    m[(r == H - 1) & (c == 0), 7] = 1.0
    m[(r == H - 1) & (c == W - 1), 8] = 1.0
    return m.reshape(NPG, PG, 9)


def _make_in_maps(inputs):
    f32 = np.float32
    shared = {
        "wconvInput": np.ascontiguousarray(inputs["wconvInput"], dtype=f32),
        "wconvHidden": np.ascontiguousarray(inputs["wconvHidden"], dtype=f32),
        "gateBias": np.ascontiguousarray(inputs["gateBias"],
                                         dtype=f32).reshape(PIX, D),
        "identity": np.eye(128, dtype=f32),
        "gap_masks": _gap_masks(),
    }
    for nm in ("wxi", "whi", "inputBias", "wxf", "whf", "forgetBias",
               "wxo", "who", "outputBias"):
        shared[nm] = np.ascontiguousarray(inputs[nm], dtype=f32).reshape(D, 1)

    xin = np.ascontiguousarray(inputs["inputs"], dtype=f32).reshape(B, PIX, CIN)
    xsp = np.ascontiguousarray(inputs["state_prev"], dtype=f32).reshape(B, PIX, D)
    xhp = np.ascontiguousarray(inputs["hidden_prev"], dtype=f32).reshape(B, PIX, D)

    in_maps = []
    for k in range(N_CORES):
        sl = slice(k * BL, (k + 1) * BL)
        m = dict(shared)
        m["inputs"] = xin[sl]
        m["state_prev"] = xsp[sl]
        m["hidden_prev"] = xhp[sl]
        m["gate_idx"] = _gate_idx(k)
        in_maps.append(m)
    return in_maps


def kernel(**inputs):
    nc = _get_nc()
    in_maps = _make_in_maps(inputs)
    res = run_bass_kernel_spmd(nc, in_maps, core_ids=list(range(N_CORES)))

    def unshard(name):
        # per-core outputs are [BL, NDC, 128, PIX] (channel-major); restore NHWC
        full = np.concatenate([res.results[k][name] for k in range(N_CORES)],
                              axis=0)
        return np.ascontiguousarray(full.transpose(0, 3, 1, 2)).reshape(
            B, H, W, D)

    return unshard("hidden"), unshard("state")



# revision 4
# speedup vs baseline: 1.6021x; 1.6021x over previous
"""Trainium2 Bass kernel for nn_DeformableConvLSTMCell_33895881900284.

Full (unsharded) inputs in, full outputs out. Data-parallel over batch across
8 NeuronCores (8 batches per core), conv weights / gate params replicated.

Math per the reference:
  outI  = conv3x3_same(inputs, wconvInput)
  g     = tanh(outI + conv3x3_same(hidden_prev, wconvHidden) + gateBias)
  gapI  = mean_hw(outI);  gapH = mean_hw(hidden_prev)          # [B, D]
  i/f/o = sigmoid(wx*gapI + wh*gapH + bias)                    # [B, D]
  tiled gate: value used at (b, h, w, c) is gate[(28*b + h) % 64, c]
  state  = f*state_prev + i*g;  hidden = o*tanh(state)

The (28*b+h)%64 scrambling makes gates cross-batch: each core computes its
local GAP columns, all cores AllGather them, and a per-core index-array input
drives an indirect-DMA gather of exactly the gate rows this core's outputs
need (the SPMD program stays identical across cores; only input data differs).

v2 layout strategy: the host pre-packs inputs/hidden/state into channel-major
bf16 ([cc, 128, pix]; inputs/hidden zero-padded to 30x30) so the device does
no layout transposes at all. The 3x3 conv is 36 shifted bf16 matmuls plus a
gateBias identity-matmul accumulating in one PSUM bank per 392-pixel window.
gapI never touches the conv output: by linearity, 784*gapI is a combination
of 9 masked pixel sums of the raw input (full sum, edge rows/cols, corners),
computed here with a handful of strided DVE reduces directly on the
channel-major tiles. The AllGather fires ~40us in; all gate-dependent work
(indirect gather, sel transposes, gate tables, elementwise) is placed late in
each engine's program so the collective never head-of-line-blocks the PE conv
stream. Elementwise for batches 0..6 is deferred until gates arrive and then
overlaps batch 7's convs.
"""
import numpy as np
import ml_dtypes

import bass_rust
import concourse.bass as bass
import concourse.mybir as mybir
import concourse.tile as tile
from concourse.bass_utils import run_bass_kernel_spmd

F32 = mybir.dt.float32
BF16 = mybir.dt.bfloat16
I32 = mybir.dt.int32
AF = mybir.ActivationFunctionType
ALU = mybir.AluOpType
AX = mybir.AxisListType

NP_BF16 = ml_dtypes.bfloat16

N_CORES = 8
B, H, W, CIN, D = 64, 28, 28, 256, 256
BL = B // N_CORES          # local batches per core
PIX = H * W                # 784
PAD = 30                   # padded row/col length
XTLEN = PAD * PAD          # 900
NW = 2                     # conv PSUM windows per batch
WROWS = H // NW            # 14
WN = WROWS * W             # 392
NCC = CIN // 128           # 2 input-channel chunks
NDC = D // 128             # 2 output-channel chunks

# batch index (in PE program order) after which the gate-dependent PE work
# (sel-table transposes) is placed
SEL_AFTER_BATCH = 6

# tap order t = 3*kh + kw ; dh = kh-1, dw = kw-1
TAPS = [(kh, kw) for kh in range(3) for kw in range(3)]

# ---------------------------------------------------------------------------
# walrus fixup: split semaphore waits that exceed the per-instruction budget
# (observed: Drain and Matmult accept only 1 semaphore wait each).
MAX_WAITS = 1


def _split_excess_sem_waits(nc):
    counter = [0]
    for fn in nc.m.functions:
        for bb in fn.blocks:
            insts = bb.instructions
            i = 0
            while i < len(insts):
                inst = insts[i]
                si = inst.sync_info
                if si is not None and si.on_wait and len(si.on_wait) > MAX_WAITS:
                    waits = list(si.on_wait)
                    excess = waits[:-MAX_WAITS]
                    keep = waits[-MAX_WAITS:]
                    new_insts = []
                    for j in range(0, len(excess), MAX_WAITS):
                        chunk = excess[j:j + MAX_WAITS]
                        noop = mybir.InstNoOp(
                            name=f"I-waitsplit-{counter[0]}", ins=[], outs=[])
                        counter[0] += 1
                        noop.engine = inst.engine
                        noop.sync_info = bass_rust.SyncInfo(
                            on_wait=chunk, on_update=[])
                        nc.register_instruction(noop)
                        new_insts.append(noop)
                    inst.sync_info = bass_rust.SyncInfo(
                        on_wait=keep, on_update=list(si.on_update))
                    insts[i:i] = new_insts
                    i += len(new_insts)
                i += 1
    return nc


# ---------------------------------------------------------------------------
def build_nc():
    nc = bass.Bass("TRN2", target_bir_lowering=False, debug=False,
                   num_devices=N_CORES)

    dram = {}
    dram["xin"] = nc.dram_tensor("xin", [BL, NCC, 128, XTLEN], BF16,
                                 kind="ExternalInput").ap()
    dram["xhid"] = nc.dram_tensor("xhid", [BL, NCC, 128, XTLEN], BF16,
                                  kind="ExternalInput").ap()
    dram["xsp"] = nc.dram_tensor("xsp", [BL, NDC, 128, PIX], BF16,
                                 kind="ExternalInput").ap()
    # conv weights pre-packed: blocks [(conv*9+t)*NCC+cc] each [128, 256]
    dram["wall"] = nc.dram_tensor("wall", [128, 2 * 9 * NCC * 256], BF16,
                                  kind="ExternalInput").ap()
    dram["gbT"] = nc.dram_tensor("gbT", [NDC, 128, PIX], BF16,
                                 kind="ExternalInput").ap()
    dram["vec"] = {}
    for nm in ("wxi", "whi", "inputBias", "wxf", "whf", "forgetBias",
               "wxo", "who", "outputBias"):
        dram["vec"][nm] = nc.dram_tensor(nm, [D, 1], F32,
                                         kind="ExternalInput").ap()
    dram["ident"] = nc.dram_tensor("identity", [128, 128], F32,
                                   kind="ExternalInput").ap()
    dram["identb"] = nc.dram_tensor("identityb", [128, 128], BF16,
                                    kind="ExternalInput").ap()
    dram["idx"] = nc.dram_tensor("gate_idx", [4, 224, 1], I32,
                                 kind="ExternalInput").ap()
    dram["hidden"] = nc.dram_tensor("hidden", [BL, NDC, 128, PIX], BF16,
                                    kind="ExternalOutput").ap()
    dram["state"] = nc.dram_tensor("state", [BL, NDC, 128, PIX], BF16,
                                   kind="ExternalOutput").ap()
    dram["cc_in"] = nc.dram_tensor("cc_in", [32, 128], F32, kind="Internal").ap()
    dram["cc_out"] = nc.dram_tensor("cc_out", [N_CORES * 32, 128], F32,
                                    kind="Internal", addr_space="Shared").ap()

    ctx_mgr = nc.allow_low_precision("bf16 datapath; tolerance 2e-2")
    ctx_mgr.__enter__()
    with tile.TileContext(nc) as tc:
        _build_body(nc, tc, dram)
    ctx_mgr.__exit__(None, None, None)
    return nc


def _build_body(nc, tc, dram):
    from contextlib import ExitStack
    ctx = ExitStack()
    pool = lambda **kw: ctx.enter_context(tc.tile_pool(**kw))

    const = pool(name="const", bufs=1)
    xbuf = pool(name="xbuf", bufs=1)      # persistent x tiles (all batches)
    gsm = pool(name="gsm", bufs=1)        # gap/gate small tiles
    gtp = pool(name="gtp", bufs=NDC * (SEL_AFTER_BATCH + 1) + 2)  # gt tiles
    ew = pool(name="ew", bufs=3)          # elementwise transients
    outb = pool(name="outb", bufs=6)      # output tiles in flight
    stg = pool(name="stg", bufs=2)        # gather staging
    ps_conv = pool(name="ps_conv", bufs=4, space="PSUM")
    ps_sm = pool(name="ps_sm", bufs=1, space="PSUM")

    # ---------------- constants ----------------
    ident = const.tile([128, 128], F32, tag="ident")
    nc.sync.dma_start(ident[:], dram["ident"][:])
    identb = const.tile([128, 128], BF16, tag="identb")
    nc.sync.dma_start(identb[:], dram["identb"][:])
    idx_sb = []
    for g4 in range(4):
        halves = []
        for hf in range(2):
            t = const.tile([112, 1], I32, tag=f"idx{g4}_{hf}")
            nc.sync.dma_start(t[:], dram["idx"][g4, hf * 112:(hf + 1) * 112, :])
            halves.append(t)
        idx_sb.append(halves)

    wall = const.tile([128, 2 * 9 * NCC * 256], BF16, tag="wall")
    nc.scalar.dma_start(wall[:], dram["wall"][:])

    def wblk(conv, t, cc):
        off = ((conv * 9 + t) * NCC + cc) * 256
        return wall[:, off:off + 256]

    gbias = [const.tile([128, PIX], BF16, tag=f"gb{dc}", name=f"gb{dc}")
             for dc in range(NDC)]
    for dc in range(NDC):
        nc.scalar.dma_start(gbias[dc][:], dram["gbT"][dc])

    vecs = {}
    for nm in dram["vec"]:
        t = const.tile([128, NDC], F32, tag=f"vec_{nm}")
        for c in range(NDC):
            nc.scalar.dma_start(t[:, c:c + 1],
                                dram["vec"][nm][c * 128:(c + 1) * 128, :])
        if nm.startswith("wx") or nm.startswith("wh"):
            nc.vector.tensor_scalar_mul(t[:], t[:], 1.0 / PIX)
        vecs[nm] = t

    # ---------------- input DMAs (all batches) ----------------
    xin = [xbuf.tile([128, BL * XTLEN], BF16, tag=f"xin{cc}", name=f"xin{cc}")
           for cc in range(NCC)]
    xhid = [xbuf.tile([128, BL * XTLEN], BF16, tag=f"xhid{cc}",
                      name=f"xhid{cc}") for cc in range(NCC)]
    xsp = [xbuf.tile([128, BL * PIX], BF16, tag=f"xsp{dc}", name=f"xsp{dc}")
           for dc in range(NDC)]
    for j in range(BL):
        for cc in range(NCC):
            nc.sync.dma_start(
                xin[cc][:, j * XTLEN:(j + 1) * XTLEN], dram["xin"][j, cc])
            nc.sync.dma_start(
                xhid[cc][:, j * XTLEN:(j + 1) * XTLEN], dram["xhid"][j, cc])
    for j in range(BL):
        for dc in range(NDC):
            nc.sync.dma_start(
                xsp[dc][:, j * PIX:(j + 1) * PIX], dram["xsp"][j, dc])

    def x3(buf, cc, j):
        """[128, 30, 30] padded view of batch j"""
        return buf[cc][:, j * XTLEN:(j + 1) * XTLEN].rearrange(
            "c (h w) -> c h w", w=PAD)

    # ---------------- stage A: masked pixel sums on DVE ----------------
    # raw group order per batch: S, R0, RL, C0, CL, K00, K0L, KL0, KLL
    # (edges/cols negated at copy time so the a-blocks are plain tap sums)
    raw = [gsm.tile([128, 9 * BL], BF16, tag=f"raw{cc}", name=f"raw{cc}")
           for cc in range(NCC)]
    gapH = [gsm.tile([128, BL], F32, tag=f"gapH{cc}", name=f"gapH{cc}")
            for cc in range(NCC)]

    for cc in range(NCC):
        # all-batch views: [128, BL, 30, 30]
        xa = xin[cc][:].rearrange("c (b h w) -> c b h w", h=PAD, w=PAD)
        inner = xa[:, :, 1:1 + H, 1:1 + W]                  # [128, BL, 28, 28]
        rows = gsm.tile([128, BL * H], BF16, tag=f"rows{cc}")
        nc.vector.tensor_reduce(
            rows[:].rearrange("c (b h) -> c b h", h=H), inner, AX.X, ALU.add)
        cols = gsm.tile([128, BL * W], BF16, tag=f"cols{cc}")
        # transposed view: [128, BL, w, h]
        colv = inner.rearrange("c b h w -> c b w h")
        nc.vector.tensor_reduce(
            cols[:].rearrange("c (b w) -> c b w", w=W), colv, AX.X, ALU.add)
        r3 = raw[cc][:].rearrange("c (g b) -> c g b", b=BL)
        # g0: full sums
        nc.vector.tensor_reduce(
            r3[:, 0, :], rows[:].rearrange("c (b h) -> c b h", h=H),
            AX.X, ALU.add)
        # g1,g2: -row0/-row27 sums ; g3,g4: -col0/-col27
        rview = rows[:].rearrange("c (b h) -> c b h", h=H)
        nc.vector.tensor_scalar_mul(
            r3[:, 1:3, :].rearrange("c g b -> c g b"),
            rview[:, :, 0:H:H - 1].rearrange("c b g -> c g b"), -1.0)
        cview = cols[:].rearrange("c (b w) -> c b w", w=W)
        nc.vector.tensor_scalar_mul(
            r3[:, 3:5, :].rearrange("c g b -> c g b"),
            cview[:, :, 0:W:W - 1].rearrange("c b g -> c g b"), -1.0)
        # g5..g8: corners (K00, K0L, KL0, KLL), positive
        corn = xa[:, :, 1:1 + H:H - 1, 1:1 + W:W - 1]       # [128, BL, 2, 2]
        nc.vector.tensor_copy(
            r3[:, 5:9, :].rearrange("c (i j) b -> c i j b", j=2),
            corn.rearrange("c b i j -> c i j b"))
        # hidden: full sums only
        ha = xhid[cc][:].rearrange("c (b h w) -> c b h w", h=PAD, w=PAD)
        hrows = gsm.tile([128, BL * H], BF16, tag=f"hrows{cc}")
        nc.vector.tensor_reduce(
            hrows[:].rearrange("c (b h) -> c b h", h=H),
            ha[:, :, 1:1 + H, 1:1 + W], AX.X, ALU.add)
        nc.vector.tensor_reduce(
            gapH[cc][:], hrows[:].rearrange("c (b h) -> c b h", h=H),
            AX.X, ALU.add)

    # combined A-blocks for gapI (conv-0 weights); groups:
    # g0 = sum of all 9 taps; g1 = taps{6,7,8}; g2 = taps{0,1,2};
    # g3 = taps{2,5,8}; g4 = taps{0,3,6}; corners g5..8 use wblk directly.
    a_r = const.tile([128, NCC * 5 * 256], BF16, tag="a_r")

    def ablk(cc, g):
        off = (cc * 5 + g) * 256
        return a_r[:, off:off + 256]

    for cc in range(NCC):
        for g, taps in ((1, [6, 7, 8]), (2, [0, 1, 2]),
                        (3, [2, 5, 8]), (4, [0, 3, 6])):
            nc.vector.tensor_tensor(out=ablk(cc, g), in0=wblk(0, taps[0], cc),
                                    in1=wblk(0, taps[1], cc), op=ALU.add)
            nc.vector.tensor_tensor(out=ablk(cc, g), in0=ablk(cc, g),
                                    in1=wblk(0, taps[2], cc), op=ALU.add)
        nc.vector.tensor_tensor(out=ablk(cc, 0), in0=ablk(cc, 1),
                                in1=ablk(cc, 2), op=ALU.add)
        for t in (3, 4, 5):
            nc.vector.tensor_tensor(out=ablk(cc, 0), in0=ablk(cc, 0),
                                    in1=wblk(0, t, cc), op=ALU.add)

    CORNER_TAP = {5: 8, 6: 6, 7: 2, 8: 0}

    # ---------------- conv pipeline ----------------
    gts = {}   # (j, dc) -> gt tile [128, PIX]

    def emit_conv_batch(j):
        for dc in range(NDC):
            gt = gtp.tile([128, PIX], BF16, tag="gt", name=f"gt{j}_{dc}")
            gts[(j, dc)] = gt
            for wi in range(NW):
                h0 = 1 + wi * WROWS
                base = (h0 - 1) * W
                p = ps_conv.tile([128, WN], F32, tag="pconv", name="pconv")
                p3 = p[:].rearrange("c (r w) -> c r w", w=W)
                nc.tensor.matmul(p[:], identb[:],
                                 gbias[dc][:, base:base + WN],
                                 start=True, stop=False)
                for conv, xb in ((0, xin), (1, xhid)):
                    for t, (kh, kw) in enumerate(TAPS):
                        dh, dw = kh - 1, kw - 1
                        for cc in range(NCC):
                            rhs = x3(xb, cc, j)[
                                :, h0 + dh:h0 + dh + WROWS, 1 + dw:1 + dw + W]
                            last = (conv == 1 and t == 8 and cc == NCC - 1)
                            nc.tensor.matmul(
                                p3, wblk(conv, t, cc)[:, dc * 128:(dc + 1) * 128],
                                rhs, start=False, stop=last)
                nc.scalar.activation(gt[:, base:base + WN], p[:], AF.Tanh)

    def emit_elementwise_batch(j, gates):
        stT = [outb.tile([128, PIX], BF16, tag="stT", name=f"stT{j}_{dc}")
               for dc in range(NDC)]
        hidT = [outb.tile([128, PIX], BF16, tag="hidT", name=f"hidT{j}_{dc}")
                for dc in range(NDC)]
        t0 = j * H
        for dc in range(NDC):
            def gw(gate):
                return gates[gate][dc][:, t0:t0 + H].to_broadcast([128, H, W])

            sp3 = xsp[dc][:, j * PIX:(j + 1) * PIX].rearrange(
                "c (h w) -> c h w", w=W)
            g3 = gts[(j, dc)][:].rearrange("c (h w) -> c h w", w=W)
            st3 = stT[dc][:].rearrange("c (h w) -> c h w", w=W)
            hd3 = hidT[dc][:].rearrange("c (h w) -> c h w", w=W)
            s1 = ew.tile([128, PIX], BF16, tag="s1", name="s1")
            s13 = s1[:].rearrange("c (h w) -> c h w", w=W)
            nc.gpsimd.tensor_tensor(out=s13, in0=sp3, in1=gw("f"), op=ALU.mult)
            s2 = ew.tile([128, PIX], BF16, tag="s2", name="s2")
            s23 = s2[:].rearrange("c (h w) -> c h w", w=W)
            nc.vector.tensor_tensor(out=s23, in0=g3, in1=gw("i"), op=ALU.mult)
            nc.vector.tensor_tensor(out=st3, in0=s13, in1=s23, op=ALU.add)
            th = ew.tile([128, PIX], BF16, tag="th", name="th")
            nc.scalar.activation(th[:], stT[dc][:], AF.Tanh)
            nc.gpsimd.tensor_tensor(out=hd3,
                                    in0=th[:].rearrange("c (h w) -> c h w", w=W),
                                    in1=gw("o"), op=ALU.mult)
            nc.scalar.dma_start(dram["state"][j, dc], stT[dc][:])
            nc.scalar.dma_start(dram["hidden"][j, dc], hidT[dc][:])

    # ---- batches 0..1 convs (PE runway before gap-combine) ----
    emit_conv_batch(0)
    emit_conv_batch(1)

    # ---- gapI combine + staging + AllGather launch ----
    gap_ps = ps_sm.tile([8, 256], F32, tag="gapI")
    first = True
    for cc in range(NCC):
        r3 = raw[cc][:].rearrange("c (g b) -> c g b", b=BL)
        for g in range(9):
            lhsT = r3[:, g, :]
            rhsA = ablk(cc, g) if g < 5 else wblk(0, CORNER_TAP[g], cc)
            nc.tensor.matmul(gap_ps[:], lhsT, rhsA,
                             start=first,
                             stop=(cc == NCC - 1 and g == 8))
            first = False
    gapI_sb = gsm.tile([8, 256], F32, tag="gapI_sb")
    nc.vector.tensor_copy(gapI_sb[:], gap_ps[:])
    nc.sync.dma_start(dram["cc_in"][0:8, :], gapI_sb[:, 0:128])
    nc.sync.dma_start(dram["cc_in"][8:16, :], gapI_sb[:, 128:256])
    for cc in range(NCC):
        pt = ps_sm.tile([8, 128], F32, tag="gapHt")
        nc.tensor.transpose(pt[:], gapH[cc][:], ident[:])
        hs = gsm.tile([8, 128], F32, tag=f"gapH_sb{cc}", name=f"gapHsb{cc}")
        nc.vector.tensor_copy(hs[:], pt[:])
        nc.sync.dma_start(dram["cc_in"][16 + 8 * cc:24 + 8 * cc, :], hs[:])

    nc.gpsimd.collective_compute(
        "AllGather", ALU.bypass, replica_groups=[list(range(N_CORES))],
        ins=[dram["cc_in"][:]], outs=[dram["cc_out"][:]])

    # ---- batches 2..SEL_AFTER_BATCH convs ----
    for j in range(2, SEL_AFTER_BATCH + 1):
        emit_conv_batch(j)

    # ---- gather + gate tables (waits on AllGather) ----
    sel = [gsm.tile([128, 224], F32, tag=f"sel{g4}", name=f"sel{g4}")
           for g4 in range(4)]
    for g4 in range(4):
        for hf in range(2):
            gtile = stg.tile([112, 128], F32, tag="gath", name="gath")
            nc.gpsimd.indirect_dma_start(
                out=gtile[:], out_offset=None, in_=dram["cc_out"][:],
                in_offset=bass.IndirectOffsetOnAxis(ap=idx_sb[g4][hf][:, :1],
                                                    axis=0))
            pt = ps_sm.tile([128, 112], F32, tag="selT", name="pt_gath")
            nc.tensor.transpose(pt[:], gtile[:], ident[0:112, 0:112])
            nc.vector.tensor_copy(sel[g4][:, hf * 112:(hf + 1) * 112], pt[:])

    gates = {}
    for gate, wx, wh, bi in (("i", "wxi", "whi", "inputBias"),
                             ("f", "wxf", "whf", "forgetBias"),
                             ("o", "wxo", "who", "outputBias")):
        per_dc = []
        for dc in range(NDC):
            t1 = gsm.tile([128, 224], F32, tag=f"gm1_{gate}{dc}")
            nc.vector.tensor_scalar_mul(t1[:], sel[dc][:],
                                        vecs[wx][:, dc:dc + 1])
            t2 = gsm.tile([128, 224], F32, tag=f"gm2_{gate}{dc}")
            nc.vector.tensor_scalar_mul(t2[:], sel[2 + dc][:],
                                        vecs[wh][:, dc:dc + 1])
            nc.vector.tensor_tensor(out=t1[:], in0=t1[:], in1=t2[:], op=ALU.add)
            gt = gsm.tile([128, 224], BF16, tag=f"gate_{gate}{dc}",
                          name=f"gate_{gate}{dc}")
            nc.scalar.activation(gt[:], t1[:], AF.Sigmoid,
                                 bias=vecs[bi][:, dc:dc + 1])
            per_dc.append(gt)
        gates[gate] = per_dc

    # ---- deferred elementwise for batches 0..SEL_AFTER_BATCH ----
    for j in range(SEL_AFTER_BATCH + 1):
        emit_elementwise_batch(j, gates)

    # ---- remaining batches: conv then elementwise ----
    for j in range(SEL_AFTER_BATCH + 1, BL):
        emit_conv_batch(j)
        emit_elementwise_batch(j, gates)

    ctx.close()


# ---------------------------------------------------------------------------
_NC_CACHE = None


def _get_nc():
    global _NC_CACHE
    if _NC_CACHE is None:
        nc = build_nc()
        _split_excess_sem_waits(nc)
        _NC_CACHE = nc
    return _NC_CACHE


def _gate_idx(core):
    idx = np.empty((4, 224, 1), np.int32)
    for j in range(BL):
        for hh in range(H):
            t = j * H + hh
            sel_b = (H * (BL * core + j) + hh) % B
            cp, bp = sel_b // BL, sel_b % BL
            for g in range(4):
                idx[g, t, 0] = cp * 32 + g * 8 + bp
    return idx


def _pack_chan_major_padded(x):
    """[B, 28, 28, 256] f32 -> [B, NCC, 128, 900] bf16 (zero-padded 30x30)"""
    xb = x.reshape(B, H, W, NCC, 128).transpose(0, 3, 4, 1, 2)
    out = np.zeros((B, NCC, 128, PAD, PAD), dtype=NP_BF16)
    out[:, :, :, 1:1 + H, 1:1 + W] = xb.astype(NP_BF16)
    return out.reshape(B, NCC, 128, XTLEN)


def _pack_chan_major(x):
    """[B, 28, 28, 256] f32 -> [B, NDC, 128, 784] bf16"""
    return np.ascontiguousarray(
        x.reshape(B, PIX, NDC, 128).transpose(0, 2, 3, 1)).astype(NP_BF16)


def _pack_weights(w1, w2):
    blocks = []
    for w in (w1, w2):
        for kh in range(3):
            for kw in range(3):
                for cc in range(NCC):
                    blocks.append(w[kh, kw, cc * 128:(cc + 1) * 128, :])
    return np.ascontiguousarray(np.concatenate(blocks, axis=1)).astype(NP_BF16)


def _make_in_maps(inputs):
    f32 = np.float32
    gb = np.asarray(inputs["gateBias"], dtype=f32).reshape(PIX, D)
    shared = {
        "wall": _pack_weights(np.asarray(inputs["wconvInput"], dtype=f32),
                              np.asarray(inputs["wconvHidden"], dtype=f32)),
        "gbT": np.ascontiguousarray(
            gb.reshape(PIX, NDC, 128).transpose(1, 2, 0)).astype(NP_BF16),
        "identity": np.eye(128, dtype=f32),
        "identityb": np.eye(128, dtype=NP_BF16),
    }
    for nm in ("wxi", "whi", "inputBias", "wxf", "whf", "forgetBias",
               "wxo", "who", "outputBias"):
        shared[nm] = np.ascontiguousarray(inputs[nm], dtype=f32).reshape(D, 1)

    xin = _pack_chan_major_padded(np.asarray(inputs["inputs"], dtype=f32))
    xhid = _pack_chan_major_padded(np.asarray(inputs["hidden_prev"], dtype=f32))
    xsp = _pack_chan_major(np.asarray(inputs["state_prev"], dtype=f32))

    in_maps = []
    for k in range(N_CORES):
        sl = slice(k * BL, (k + 1) * BL)
        m = dict(shared)
        m["xin"] = xin[sl]
        m["xhid"] = xhid[sl]
        m["xsp"] = xsp[sl]
        m["gate_idx"] = _gate_idx(k)
        in_maps.append(m)
    return in_maps


def kernel(**inputs):
    nc = _get_nc()
    in_maps = _make_in_maps(inputs)
    res = run_bass_kernel_spmd(nc, in_maps, core_ids=list(range(N_CORES)))

    def unshard(name):
        # per-core outputs are [BL, NDC, 128, PIX] (channel-major); restore NHWC
        full = np.concatenate([res.results[k][name] for k in range(N_CORES)],
                              axis=0)
        return np.ascontiguousarray(
            full.astype(np.float32).transpose(0, 3, 1, 2)).reshape(B, H, W, D)

    return unshard("hidden"), unshard("state")


# revision 17
# speedup vs baseline: 1.8081x; 1.1286x over previous
"""Trainium2 Bass kernel for nn_DeformableConvLSTMCell_33895881900284.

Full (unsharded) inputs in, full outputs out. Data-parallel over batch across
8 NeuronCores (8 batches per core), conv weights / gate params replicated.

Math per the reference:
  outI  = conv3x3_same(inputs, wconvInput)
  g     = tanh(outI + conv3x3_same(hidden_prev, wconvHidden) + gateBias)
  gapI  = mean_hw(outI);  gapH = mean_hw(hidden_prev)          # [B, D]
  i/f/o = sigmoid(wx*gapI + wh*gapH + bias)                    # [B, D]
  tiled gate: value used at (b, h, w, c) is gate[(28*b + h) % 64, c]
  state  = f*state_prev + i*g;  hidden = o*tanh(state)

The (28*b+h)%64 scrambling makes gates cross-batch: each core computes its
local GAP columns, all cores AllGather them, and a per-core index-array input
drives an indirect-DMA gather of exactly the gate rows this core's outputs
need (the SPMD program stays identical across cores; only input data differs).

v2 layout strategy: the host pre-packs inputs/hidden/state into channel-major
bf16 ([cc, 128, pix]; inputs/hidden zero-padded to 30x30) so the device does
no layout transposes at all. The 3x3 conv is 36 shifted bf16 matmuls plus a
gateBias identity-matmul accumulating in one PSUM bank per 392-pixel window.
gapI never touches the conv output: by linearity, 784*gapI is a combination
of 9 masked pixel sums of the raw input (full sum, edge rows/cols, corners),
computed here with a handful of strided DVE reduces directly on the
channel-major tiles. The AllGather fires ~40us in; all gate-dependent work
(indirect gather, sel transposes, gate tables, elementwise) is placed late in
each engine's program so the collective never head-of-line-blocks the PE conv
stream. Elementwise for batches 0..6 is deferred until gates arrive and then
overlaps batch 7's convs.
"""
import numpy as np
import ml_dtypes

import bass_rust
import concourse.bass as bass
import concourse.mybir as mybir
import concourse.tile as tile
from concourse.bass_utils import run_bass_kernel_spmd

F32 = mybir.dt.float32
BF16 = mybir.dt.bfloat16
I32 = mybir.dt.int32
AF = mybir.ActivationFunctionType
ALU = mybir.AluOpType
AX = mybir.AxisListType

NP_BF16 = ml_dtypes.bfloat16

N_CORES = 8
B, H, W, CIN, D = 64, 28, 28, 256, 256
BL = B // N_CORES          # local batches per core
PIX = H * W                # 784
PAD = 30                   # padded row/col length
XTLEN = PAD * PAD          # 900
NW = 2                     # conv PSUM windows per batch
WROWS = H // NW            # 14
WN = WROWS * W             # 392
NCC = CIN // 128           # 2 input-channel chunks
NDC = D // 128             # 2 output-channel chunks

# batch index (in PE program order) after which the gate-dependent PE work
# (sel-table transposes) is placed
SEL_AFTER_BATCH = 2

# tap order t = 3*kh + kw ; dh = kh-1, dw = kw-1
TAPS = [(kh, kw) for kh in range(3) for kw in range(3)]

# ---------------------------------------------------------------------------
# walrus fixup: split semaphore waits that exceed the per-instruction budget
# (observed: Drain and Matmult accept only 1 semaphore wait each).
MAX_WAITS = 1


def _split_excess_sem_waits(nc):
    counter = [0]
    for fn in nc.m.functions:
        for bb in fn.blocks:
            insts = bb.instructions
            i = 0
            while i < len(insts):
                inst = insts[i]
                si = inst.sync_info
                if si is not None and si.on_wait and len(si.on_wait) > MAX_WAITS:
                    waits = list(si.on_wait)
                    excess = waits[:-MAX_WAITS]
                    keep = waits[-MAX_WAITS:]
                    new_insts = []
                    for j in range(0, len(excess), MAX_WAITS):
                        chunk = excess[j:j + MAX_WAITS]
                        noop = mybir.InstNoOp(
                            name=f"I-waitsplit-{counter[0]}", ins=[], outs=[])
                        counter[0] += 1
                        noop.engine = inst.engine
                        noop.sync_info = bass_rust.SyncInfo(
                            on_wait=chunk, on_update=[])
                        nc.register_instruction(noop)
                        new_insts.append(noop)
                    inst.sync_info = bass_rust.SyncInfo(
                        on_wait=keep, on_update=list(si.on_update))
                    insts[i:i] = new_insts
                    i += len(new_insts)
                i += 1
    return nc


# ---------------------------------------------------------------------------
def build_nc():
    nc = bass.Bass("TRN2", target_bir_lowering=False, debug=False,
                   num_devices=N_CORES)

    dram = {}
    dram["xin"] = nc.dram_tensor("xin", [BL, NCC, 128, XTLEN], BF16,
                                 kind="ExternalInput").ap()
    dram["xhid"] = nc.dram_tensor("xhid", [BL, NCC, 128, XTLEN], BF16,
                                  kind="ExternalInput").ap()
    dram["xsp"] = nc.dram_tensor("xsp", [BL, NDC, 128, PIX], BF16,
                                 kind="ExternalInput").ap()
    # conv weights pre-packed: blocks [(conv*9+t)*NCC+cc] each [128, 256]
    dram["wall"] = nc.dram_tensor("wall", [128, 2 * 9 * NCC * 256], BF16,
                                  kind="ExternalInput").ap()
    dram["gbT"] = nc.dram_tensor("gbT", [NDC, 128, PIX], F32,
                                 kind="ExternalInput").ap()
    dram["vec"] = {}
    for nm in ("wxi", "whi", "inputBias", "wxf", "whf", "forgetBias",
               "wxo", "who", "outputBias"):
        dram["vec"][nm] = nc.dram_tensor(nm, [D, 1], F32,
                                         kind="ExternalInput").ap()
    dram["ident"] = nc.dram_tensor("identity", [128, 128], F32,
                                   kind="ExternalInput").ap()
    dram["idx"] = nc.dram_tensor("gate_idx", [4, 224, 1], I32,
                                 kind="ExternalInput").ap()
    dram["hidden"] = nc.dram_tensor("hidden", [BL, NDC, 128, PIX], BF16,
                                    kind="ExternalOutput").ap()
    dram["state"] = nc.dram_tensor("state", [BL, NDC, 128, PIX], BF16,
                                   kind="ExternalOutput").ap()
    dram["cc_in"] = nc.dram_tensor("cc_in", [32, 128], F32, kind="Internal").ap()
    dram["cc_out"] = nc.dram_tensor("cc_out", [N_CORES * 32, 128], F32,
                                    kind="Internal", addr_space="Shared").ap()

    ctx_mgr = nc.allow_low_precision("bf16 datapath; tolerance 2e-2")
    ctx_mgr.__enter__()
    with tile.TileContext(nc) as tc:
        _build_body(nc, tc, dram)
    ctx_mgr.__exit__(None, None, None)
    return nc


def _build_body(nc, tc, dram):
    from contextlib import ExitStack
    ctx = ExitStack()
    pool = lambda **kw: ctx.enter_context(tc.tile_pool(**kw))

    const = pool(name="const", bufs=1)
    xbuf = pool(name="xbuf", bufs=1)      # persistent x tiles (per batch)
    gsm = pool(name="gsm", bufs=1)        # gap/gate small tiles
    gtp = pool(name="gtp", bufs=NDC * (SEL_AFTER_BATCH + 1) + 2)  # gt tiles
    ew = pool(name="ew", bufs=3)          # elementwise transients
    outb = pool(name="outb", bufs=6)      # output tiles in flight
    stg = pool(name="stg", bufs=2)        # gather staging
    trash = pool(name="trash", bufs=2)    # ACT accum side-output sink
    ps_conv = pool(name="ps_conv", bufs=4, space="PSUM")
    ps_sm = pool(name="ps_sm", bufs=1, space="PSUM")

    # ---------------- constants ----------------
    ident = const.tile([128, 128], F32, tag="ident")
    nc.sync.dma_start(ident[:], dram["ident"][:])
    idx_sb = []
    for g4 in range(4):
        halves = []
        for hf in range(2):
            t = const.tile([112, 1], I32, tag=f"idx{g4}_{hf}")
            nc.sync.dma_start(t[:], dram["idx"][g4, hf * 112:(hf + 1) * 112, :])
            halves.append(t)
        idx_sb.append(halves)

    wall = const.tile([128, 2 * 9 * NCC * 256], BF16, tag="wall")
    nc.scalar.dma_start(wall[:], dram["wall"][:])

    def wblk(conv, t, cc):
        off = ((conv * 9 + t) * NCC + cc) * 256
        return wall[:, off:off + 256]

    gbias = [const.tile([128, PIX], F32, tag=f"gb{dc}", name=f"gb{dc}")
             for dc in range(NDC)]
    for dc in range(NDC):
        nc.scalar.dma_start(gbias[dc][:], dram["gbT"][dc])

    vecs = {}
    for nm in dram["vec"]:
        t = const.tile([128, NDC], F32, tag=f"vec_{nm}")
        for c in range(NDC):
            nc.scalar.dma_start(t[:, c:c + 1],
                                dram["vec"][nm][c * 128:(c + 1) * 128, :])
        if nm.startswith("wx") or nm.startswith("wh"):
            nc.vector.tensor_scalar_mul(t[:], t[:], 1.0 / PIX)
        vecs[nm] = t

    # ---------------- input DMAs (per-batch tiles) ----------------
    xin = {}
    xhid = {}
    xsp = {}
    for j in range(BL):
        for cc in range(NCC):
            xin[(cc, j)] = xbuf.tile([128, XTLEN], BF16, tag=f"xin{cc}_{j}",
                                     name=f"xin{cc}_{j}")
            nc.sync.dma_start(xin[(cc, j)][:], dram["xin"][j, cc])
            xhid[(cc, j)] = xbuf.tile([128, XTLEN], BF16, tag=f"xhid{cc}_{j}",
                                      name=f"xhid{cc}_{j}")
            nc.sync.dma_start(xhid[(cc, j)][:], dram["xhid"][j, cc])
    for j in range(BL):
        for dc in range(NDC):
            xsp[(dc, j)] = xbuf.tile([128, PIX], BF16, tag=f"xsp{dc}_{j}",
                                     name=f"xsp{dc}_{j}")
            nc.scalar.dma_start(xsp[(dc, j)][:], dram["xsp"][j, dc])

    def x3(buf, cc, j):
        """[128, 30, 30] padded view of batch j"""
        return buf[(cc, j)][:].rearrange("c (h w) -> c h w", w=PAD)

    # ---------------- stage A: masked pixel sums ----------------
    # raw group order per batch: S, R0, RL, C0, CL, K00, K0L, KL0, KLL
    # (edges negated at reduce time so the a-blocks are plain tap sums)
    raw = [gsm.tile([128, 9 * BL], F32, tag=f"raw{cc}", name=f"raw{cc}")
           for cc in range(NCC)]
    rawb = [gsm.tile([128, 9 * BL], BF16, tag=f"rawb{cc}", name=f"rawb{cc}")
            for cc in range(NCC)]
    gapH = [gsm.tile([128, BL], F32, tag=f"gapH{cc}", name=f"gapH{cc}")
            for cc in range(NCC)]

    for j in range(BL):
        for cc in range(NCC):
            r3 = raw[cc][:].rearrange("c (g b) -> c g b", b=BL)
            # full sums via ACT accumulate (padding zeros don't change sums)
            sink = trash.tile([128, XTLEN], BF16, tag="sink", name="sink")
            nc.scalar.activation(sink[:], xin[(cc, j)][:], AF.Copy,
                                 accum_out=r3[:, 0:1, j:j + 1])
            sinkh = trash.tile([128, XTLEN], BF16, tag="sinkh", name="sinkh")
            nc.scalar.activation(sinkh[:], xhid[(cc, j)][:], AF.Copy,
                                 accum_out=gapH[cc][:, j:j + 1])
            # edge rows/cols (negated)
            xa = x3(xin, cc, j)
            nc.vector.tensor_reduce(r3[:, 1:2, j:j + 1], xa[:, 1:2, 1:1 + W],
                                    AX.X, ALU.add, negate=True)
            nc.vector.tensor_reduce(r3[:, 2:3, j:j + 1], xa[:, H:H + 1, 1:1 + W],
                                    AX.X, ALU.add, negate=True)
            nc.vector.tensor_reduce(
                r3[:, 3:4, j:j + 1], xa[:, 1:1 + H, 1:2].rearrange("c h w -> c w h"),
                AX.X, ALU.add, negate=True)
            nc.vector.tensor_reduce(
                r3[:, 4:5, j:j + 1],
                xa[:, 1:1 + H, W:W + 1].rearrange("c h w -> c w h"),
                AX.X, ALU.add, negate=True)
            # corners K00, K0L, KL0, KLL (positive)
            corn = xa[:, 1:1 + H:H - 1, 1:1 + W:W - 1]      # [128, 2, 2]
            nc.vector.tensor_copy(
                r3[:, 5:9, j:j + 1].rearrange("c (i k) b -> c i (k b)", k=2),
                corn)
    for cc in range(NCC):
        nc.vector.tensor_copy(rawb[cc][:], raw[cc][:])

    # combined A-blocks for gapI (conv-0 weights); groups:
    # g0 = sum of all 9 taps; g1 = taps{6,7,8}; g2 = taps{0,1,2};
    # g3 = taps{2,5,8}; g4 = taps{0,3,6}; corners g5..8 use wblk directly.
    a_r = const.tile([128, NCC * 5 * 256], BF16, tag="a_r")

    def ablk(cc, g):
        off = (cc * 5 + g) * 256
        return a_r[:, off:off + 256]

    for cc in range(NCC):
        for g, taps in ((1, [6, 7, 8]), (2, [0, 1, 2]),
                        (3, [2, 5, 8]), (4, [0, 3, 6])):
            nc.vector.tensor_tensor(out=ablk(cc, g), in0=wblk(0, taps[0], cc),
                                    in1=wblk(0, taps[1], cc), op=ALU.add)
            nc.vector.tensor_tensor(out=ablk(cc, g), in0=ablk(cc, g),
                                    in1=wblk(0, taps[2], cc), op=ALU.add)
        nc.vector.tensor_tensor(out=ablk(cc, 0), in0=ablk(cc, 1),
                                in1=ablk(cc, 2), op=ALU.add)
        for t in (3, 4, 5):
            nc.vector.tensor_tensor(out=ablk(cc, 0), in0=ablk(cc, 0),
                                    in1=wblk(0, t, cc), op=ALU.add)

    CORNER_TAP = {5: 8, 6: 6, 7: 2, 8: 0}

    # ---------------- conv pipeline ----------------
    gts = {}   # (j, dc) -> gt tile [128, PIX]

    def emit_conv_batch(j):
        for dc in range(NDC):
            gt = gtp.tile([128, PIX], BF16, tag="gt", name=f"gt{j}_{dc}")
            gts[(j, dc)] = gt
            for wi in range(NW):
                h0 = 1 + wi * WROWS
                base = (h0 - 1) * W
                p = ps_conv.tile([128, WN], F32, tag="pconv", name="pconv")
                p3 = p[:].rearrange("c (r w) -> c r w", w=W)
                first = True
                for conv, xb in ((0, xin), (1, xhid)):
                    for t, (kh, kw) in enumerate(TAPS):
                        dh, dw = kh - 1, kw - 1
                        for cc in range(NCC):
                            rhs = x3(xb, cc, j)[
                                :, h0 + dh:h0 + dh + WROWS, 1 + dw:1 + dw + W]
                            last = (conv == 1 and t == 8 and cc == NCC - 1)
                            nc.tensor.matmul(
                                p3, wblk(conv, t, cc)[:, dc * 128:(dc + 1) * 128],
                                rhs, start=first, stop=last)
                            first = False
                # += gateBias, then tanh (keeps the PE stream pure matmuls)
                tmp = ew.tile([128, WN], F32, tag="cbias", name="cbias")
                nc.vector.tensor_tensor(out=tmp[:], in0=p[:],
                                        in1=gbias[dc][:, base:base + WN],
                                        op=ALU.add)
                nc.scalar.activation(gt[:, base:base + WN], tmp[:], AF.Tanh)

    def emit_elementwise_batch(j, gates):
        stT = [outb.tile([128, PIX], BF16, tag="stT", name=f"stT{j}_{dc}")
               for dc in range(NDC)]
        hidT = [outb.tile([128, PIX], BF16, tag="hidT", name=f"hidT{j}_{dc}")
                for dc in range(NDC)]
        t0 = j * H
        for dc in range(NDC):
            def gw(gate):
                return gates[gate][dc][:, t0:t0 + H].to_broadcast([128, H, W])

            sp3 = xsp[(dc, j)][:].rearrange("c (h w) -> c h w", w=W)
            g3 = gts[(j, dc)][:].rearrange("c (h w) -> c h w", w=W)
            st3 = stT[dc][:].rearrange("c (h w) -> c h w", w=W)
            hd3 = hidT[dc][:].rearrange("c (h w) -> c h w", w=W)
            s1 = ew.tile([128, PIX], BF16, tag="s1", name="s1")
            s13 = s1[:].rearrange("c (h w) -> c h w", w=W)
            nc.gpsimd.tensor_tensor(out=s13, in0=sp3, in1=gw("f"), op=ALU.mult)
            s2 = ew.tile([128, PIX], BF16, tag="s2", name="s2")
            s23 = s2[:].rearrange("c (h w) -> c h w", w=W)
            nc.vector.tensor_tensor(out=s23, in0=g3, in1=gw("i"), op=ALU.mult)
            nc.vector.tensor_tensor(out=st3, in0=s13, in1=s23, op=ALU.add)
            th = ew.tile([128, PIX], BF16, tag="th", name="th")
            nc.scalar.activation(th[:], stT[dc][:], AF.Tanh)
            heng = nc.gpsimd if dc == 0 else nc.vector
            heng.tensor_tensor(out=hd3,
                               in0=th[:].rearrange("c (h w) -> c h w", w=W),
                               in1=gw("o"), op=ALU.mult)
            nc.scalar.dma_start(dram["state"][j, dc], stT[dc][:])
            nc.scalar.dma_start(dram["hidden"][j, dc], hidT[dc][:])

    # ---- batch 0 convs (PE runway before gap-combine) ----
    emit_conv_batch(0)

    # ---- gapI combine + staging + AllGather launch ----
    gap_ps = ps_sm.tile([8, 256], F32, tag="gapI")
    first = True
    for cc in range(NCC):
        r3 = rawb[cc][:].rearrange("c (g b) -> c g b", b=BL)
        for g in range(9):
            lhsT = r3[:, g, :]
            rhsA = ablk(cc, g) if g < 5 else wblk(0, CORNER_TAP[g], cc)
            nc.tensor.matmul(gap_ps[:], lhsT, rhsA,
                             start=first,
                             stop=(cc == NCC - 1 and g == 8))
            first = False
    gapI_sb = gsm.tile([8, 256], F32, tag="gapI_sb")
    nc.vector.tensor_copy(gapI_sb[:], gap_ps[:])
    nc.sync.dma_start(dram["cc_in"][0:8, :], gapI_sb[:, 0:128])
    nc.sync.dma_start(dram["cc_in"][8:16, :], gapI_sb[:, 128:256])
    for cc in range(NCC):
        pt = ps_sm.tile([8, 128], F32, tag="gapHt")
        nc.tensor.transpose(pt[:], gapH[cc][:], ident[:])
        hs = gsm.tile([8, 128], F32, tag=f"gapH_sb{cc}", name=f"gapHsb{cc}")
        nc.vector.tensor_copy(hs[:], pt[:])
        nc.sync.dma_start(dram["cc_in"][16 + 8 * cc:24 + 8 * cc, :], hs[:])

    nc.gpsimd.collective_compute(
        "AllGather", ALU.bypass, replica_groups=[list(range(N_CORES))],
        ins=[dram["cc_in"][:]], outs=[dram["cc_out"][:]])

    # ---- batches 1..SEL_AFTER_BATCH convs ----
    for j in range(1, SEL_AFTER_BATCH + 1):
        emit_conv_batch(j)

    # ---- gather + gate tables (waits on AllGather) ----
    sel = [gsm.tile([128, 224], F32, tag=f"sel{g4}", name=f"sel{g4}")
           for g4 in range(4)]
    for g4 in range(4):
        for hf in range(2):
            gtile = stg.tile([112, 128], F32, tag="gath", name="gath")
            nc.gpsimd.indirect_dma_start(
                out=gtile[:], out_offset=None, in_=dram["cc_out"][:],
                in_offset=bass.IndirectOffsetOnAxis(ap=idx_sb[g4][hf][:, :1],
                                                    axis=0))
            pt = ps_sm.tile([128, 112], F32, tag="selT", name="pt_gath")
            nc.tensor.transpose(pt[:], gtile[:], ident[0:112, 0:112])
            nc.vector.tensor_copy(sel[g4][:, hf * 112:(hf + 1) * 112], pt[:])

    gates = {}
    for gate, wx, wh, bi in (("i", "wxi", "whi", "inputBias"),
                             ("f", "wxf", "whf", "forgetBias"),
                             ("o", "wxo", "who", "outputBias")):
        per_dc = []
        for dc in range(NDC):
            t1 = gsm.tile([128, 224], F32, tag=f"gm1_{gate}{dc}")
            nc.vector.tensor_scalar_mul(t1[:], sel[dc][:],
                                        vecs[wx][:, dc:dc + 1])
            t2 = gsm.tile([128, 224], F32, tag=f"gm2_{gate}{dc}")
            nc.vector.tensor_scalar_mul(t2[:], sel[2 + dc][:],
                                        vecs[wh][:, dc:dc + 1])
            nc.vector.tensor_tensor(out=t1[:], in0=t1[:], in1=t2[:], op=ALU.add)
            gt = gsm.tile([128, 224], BF16, tag=f"gate_{gate}{dc}",
                          name=f"gate_{gate}{dc}")
            nc.scalar.activation(gt[:], t1[:], AF.Sigmoid,
                                 bias=vecs[bi][:, dc:dc + 1])
            per_dc.append(gt)
        gates[gate] = per_dc

    # ---- deferred elementwise for batches 0..SEL_AFTER_BATCH ----
    for j in range(SEL_AFTER_BATCH + 1):
        emit_elementwise_batch(j, gates)

    # ---- remaining batches: conv then elementwise ----
    for j in range(SEL_AFTER_BATCH + 1, BL):
        emit_conv_batch(j)
        emit_elementwise_batch(j, gates)

    ctx.close()


# ---------------------------------------------------------------------------
_NC_CACHE = None


def _get_nc():
    global _NC_CACHE
    if _NC_CACHE is None:
        nc = build_nc()
        _split_excess_sem_waits(nc)
        _NC_CACHE = nc
    return _NC_CACHE


def _gate_idx(core):
    idx = np.empty((4, 224, 1), np.int32)
    for j in range(BL):
        for hh in range(H):
            t = j * H + hh
            sel_b = (H * (BL * core + j) + hh) % B
            cp, bp = sel_b // BL, sel_b % BL
            for g in range(4):
                idx[g, t, 0] = cp * 32 + g * 8 + bp
    return idx


def _pack_chan_major_padded(x):
    """[B, 28, 28, 256] f32 -> [B, NCC, 128, 900] bf16 (zero-padded 30x30)"""
    xb = x.reshape(B, H, W, NCC, 128).transpose(0, 3, 4, 1, 2)
    out = np.zeros((B, NCC, 128, PAD, PAD), dtype=NP_BF16)
    out[:, :, :, 1:1 + H, 1:1 + W] = xb.astype(NP_BF16)
    return out.reshape(B, NCC, 128, XTLEN)


def _pack_chan_major(x):
    """[B, 28, 28, 256] f32 -> [B, NDC, 128, 784] bf16"""
    return np.ascontiguousarray(
        x.reshape(B, PIX, NDC, 128).transpose(0, 2, 3, 1)).astype(NP_BF16)


def _pack_weights(w1, w2):
    blocks = []
    for w in (w1, w2):
        for kh in range(3):
            for kw in range(3):
                for cc in range(NCC):
                    blocks.append(w[kh, kw, cc * 128:(cc + 1) * 128, :])
    return np.ascontiguousarray(np.concatenate(blocks, axis=1)).astype(NP_BF16)


def _make_in_maps(inputs):
    f32 = np.float32
    gb = np.asarray(inputs["gateBias"], dtype=f32).reshape(PIX, D)
    shared = {
        "wall": _pack_weights(np.asarray(inputs["wconvInput"], dtype=f32),
                              np.asarray(inputs["wconvHidden"], dtype=f32)),
        "gbT": np.ascontiguousarray(
            gb.reshape(PIX, NDC, 128).transpose(1, 2, 0)).astype(f32),
        "identity": np.eye(128, dtype=f32),
    }
    for nm in ("wxi", "whi", "inputBias", "wxf", "whf", "forgetBias",
               "wxo", "who", "outputBias"):
        shared[nm] = np.ascontiguousarray(inputs[nm], dtype=f32).reshape(D, 1)

    xin = _pack_chan_major_padded(np.asarray(inputs["inputs"], dtype=f32))
    xhid = _pack_chan_major_padded(np.asarray(inputs["hidden_prev"], dtype=f32))
    xsp = _pack_chan_major(np.asarray(inputs["state_prev"], dtype=f32))

    in_maps = []
    for k in range(N_CORES):
        sl = slice(k * BL, (k + 1) * BL)
        m = dict(shared)
        m["xin"] = xin[sl]
        m["xhid"] = xhid[sl]
        m["xsp"] = xsp[sl]
        m["gate_idx"] = _gate_idx(k)
        in_maps.append(m)
    return in_maps


def kernel(**inputs):
    nc = _get_nc()
    in_maps = _make_in_maps(inputs)
    res = run_bass_kernel_spmd(nc, in_maps, core_ids=list(range(N_CORES)))

    def unshard(name):
        # per-core outputs are [BL, NDC, 128, PIX] (channel-major); restore NHWC
        full = np.concatenate([res.results[k][name] for k in range(N_CORES)],
                              axis=0)
        return np.ascontiguousarray(
            full.astype(np.float32).transpose(0, 3, 1, 2)).reshape(B, H, W, D)

    return unshard("hidden"), unshard("state")
